# revision 19
# baseline (speedup 1.0000x reference)
"""MoE ExpertGroup kernel for Trainium2 (8 NeuronCores, expert-parallel).

Problem: E=8 experts, H=1024, I=4096, N=16384 tokens sorted by expert.
y[t] = gelu_tanh(x[t] @ w1[e(t)]) @ w2[e(t)]

Sharding: expert-parallel — core e holds expert e's weights and processes
expert e's contiguous token block (balanced routing: 2048 tokens/core).

v3: fp32r matmuls (measured 227ns/MM pace vs 259 for bf16 — the bf16
separate-LDWEIGHTS path serializes ~46ns/MM while fp32r's internal
weight load overlaps).  Structural wins vs the original baseline:
  - host-packed k-major DRAM layouts ([128, ktile, cols]) so each w1/w2
    group chunk and x half-chunk is ONE dma_start (52 total vs 240);
    priority-ordered so the first MM1 chain is fed after ~4MB.
  - warmup matmul count tuned to cover the first-wave DMA (HAM gate).
  - MM2 interleaves the two H-half accumulation chains so consecutive
    matmuls share the same stationary hT tile.
  - y written out per (token-tile, H-half) right after its last add.

Per-core dataflow per half (2 halves x 1024 tokens), per group g (8
groups x 4 I-tiles), all matmuls fp32r:
  MM1: ph[128 I, 512 tok] += w1c[:,k,il]^T @ xT[:,k,tb]  (k=0..7)
       gelu -> hT[il]                                     (il=0..3)
  MM2: py[hh][128 tok, 512 H] += hT[il][:,tc]^T @ w2c[:,il,hh]
       (il chains for hh=0,1 interleaved); DVE-accumulate into ysb
  last group: ysb[tt][:,hh] -> DMA out
"""

import sys

sys.path.insert(0, "/opt/trn_rl_repo")

import numpy as np
import ml_dtypes

# --- problem constants (hardcoded; kernel.py must be self-contained) ---
E = 8          # experts == cores
H = 1024       # hidden
I = 4096       # intermediate
N_TOK = 16384  # total tokens
T = N_TOK // E  # tokens per core (capacity)

P = 128
NH = 2               # token halves per core
TH = T // NH         # tokens per half (1024)
TB = 512             # token block (psum free dim)
NTB = TH // TB       # 2
KH = H // P          # 8
IB = I // P          # 32
GI = 4               # I-tiles per PSUM-accumulation group
NG = IB // GI        # 8 groups
N_WARM = 32          # PE warmup matmuls (cover first-wave DMA)

_CACHE = {}


def _build():
    import concourse.bacc as bacc
    import concourse.mybir as mybir
    import concourse.tile as tile

    F32 = mybir.dt.float32
    F32R = mybir.dt.float32r
    BF16 = mybir.dt.bfloat16
    GELU = mybir.ActivationFunctionType.Gelu_apprx_tanh

    nc = bacc.Bacc("TRN2", target_bir_lowering=False, debug=False, num_devices=E)

    # host-packed layouts: [128 partitions, k-tile, cols]
    xd = nc.dram_tensor("xT", [P, KH, T], F32R, kind="ExternalInput").ap()
    w1d = nc.dram_tensor("w1", [P, KH, I], F32R, kind="ExternalInput").ap()
    w2d = nc.dram_tensor("w2", [P, IB, H], F32R, kind="ExternalInput").ap()
    y = nc.dram_tensor("y", [T, H], F32, kind="ExternalOutput").ap()

    with tile.TileContext(nc) as tc:
        with (
            tc.tile_pool(name="wsrc", bufs=1) as wsrc_pool,
            tc.tile_pool(name="xp", bufs=1) as x_pool,
            tc.tile_pool(name="ysb", bufs=1) as y_pool,
            tc.tile_pool(name="w1p", bufs=2) as w1_pool,
            tc.tile_pool(name="w2p", bufs=2) as w2_pool,
            tc.tile_pool(name="hT", bufs=8) as hT_pool,
            tc.tile_pool(name="ph", bufs=4, space="PSUM") as ph_pool,
            tc.tile_pool(name="py", bufs=4, space="PSUM") as py_pool,
        ):
            # PE warmup: release the HAM clock gate while the first DMAs land.
            # bf16 single-pass matmuls (fp32 warmups are 2-pass LOW/HIGH and
            # eat ~860ns each); the source is memset on gpsimd.
            wsrc = wsrc_pool.tile([P, TB], BF16, tag="warm", name="wsrc")
            nc.gpsimd.memset(wsrc[:], 0.0)
            for _ in range(N_WARM):
                pw = ph_pool.tile([P, TB], F32, tag="ph", name="pw")
                nc.tensor.matmul(pw[:], wsrc[:, :P], wsrc[:], start=True, stop=True)

            COPY = mybir.ActivationFunctionType.Copy

            def fetch_w(g, pace_src=None):
                # one dma per w1 group chunk ([128, 8, 512] = 2MB)
                w1t = w1_pool.tile([P, KH, GI * P], F32R, tag="w1c", name="w1c")
                if pace_src is not None:
                    # tiny ACT copy creates a WAW dep: the transfer only
                    # starts once pace_src exists (keeps HBM bandwidth on
                    # the startup-critical tiles)
                    nc.scalar.activation(w1t[:, 0, 0:4], pace_src, COPY)
                nc.sync.dma_start(
                    out=w1t[:], in_=w1d[:, :, g * GI * P : (g + 1) * GI * P]
                )
                # one dma per w2 group chunk ([128, 4, 1024] = 2MB)
                w2t = w2_pool.tile([P, GI, H], F32R, tag="w2c", name="w2c")
                if pace_src is not None:
                    nc.scalar.activation(w2t[:, 0, 0:4], pace_src, COPY)
                nc.sync.dma_start(out=w2t[:], in_=w2d[:, g * GI : (g + 1) * GI, :])
                return w1t, w2t

            first_ht = [None]

            for half in range(NH):
                t0 = half * TH

                # priority order: the first MM1 chain needs w1 g0 cols il0
                # and xT chunk a — those transfer first; everything else is
                # paced behind them via tiny-copy WAW deps.
                xt = x_pool.tile([P, KH, TH], F32R, tag=f"xT{half}", name=f"xT{half}")
                w1t0 = w1_pool.tile([P, KH, GI * P], F32R, tag="w1c", name="w1c")
                nc.sync.dma_start(out=w1t0[:, :, 0:256], in_=w1d[:, :, 0:256])
                nc.sync.dma_start(
                    out=xt[:, :, 0:TB], in_=xd[:, :, t0 : t0 + TB]
                )
                nc.sync.dma_start(out=w1t0[:, :, 256:512], in_=w1d[:, :, 256:512])
                w2t0 = w2_pool.tile([P, GI, H], F32R, tag="w2c", name="w2c")
                if half == 0:
                    # pace w2 g0 and xT chunk b on the arrival of xT chunk a
                    nc.scalar.activation(w2t0[:, 0, 0:4], xt[:, 0, 0:4], COPY)
                    nc.scalar.activation(
                        xt[:, 0, TB : TB + 4], xt[:, 0, 4:8], COPY
                    )
                nc.sync.dma_start(out=w2t0[:], in_=w2d[:, 0:GI, :])
                nc.sync.dma_start(
                    out=xt[:, :, TB : 2 * TB], in_=xd[:, :, t0 + TB : t0 + 2 * TB]
                )

                ysb = [
                    y_pool.tile([P, H], F32, tag=f"yt{tt}", name=f"yt{tt}")
                    for tt in range(TH // P)
                ]

                for g in range(NG):
                    if g == 0:
                        w1t, w2t = w1t0, w2t0
                    else:
                        # pace the g=1 fetch of half 0 on the first gelu
                        pace = first_ht[0] if (half == 0 and g == 1) else None
                        w1t, w2t = fetch_w(g, pace_src=pace)

                    for tb in range(NTB):
                        ts_ = slice(tb * TB, (tb + 1) * TB)
                        hTt = []
                        for il in range(GI):
                            ph = ph_pool.tile([P, TB], F32, tag="ph", name="ph")
                            for k in range(KH):
                                nc.tensor.matmul(
                                    ph[:],
                                    w1t[:, k, il * P : (il + 1) * P],
                                    xt[:, k, ts_],
                                    start=(k == 0),
                                    stop=(k == KH - 1),
                                )
                            ht = hT_pool.tile([P, TB], F32R, tag="ht", name="ht")
                            nc.scalar.activation(ht[:], ph[:], GELU)
                            if first_ht[0] is None:
                                first_ht[0] = ht[:, 0:4]
                            hTt.append(ht)
                        for tc_ in range(TB // P):
                            tt = tb * (TB // P) + tc_
                            # interleave the two H-half chains: consecutive
                            # matmuls share the same stationary hT slice
                            py0 = py_pool.tile([P, H // 2], F32, tag="py", name="py")
                            py1 = py_pool.tile([P, H // 2], F32, tag="py", name="py")
                            for il in range(GI):
                                lhs = hTt[il][:, tc_ * P : (tc_ + 1) * P]
                                nc.tensor.matmul(
                                    py0[:], lhs, w2t[:, il, 0 : H // 2],
                                    start=(il == 0), stop=(il == GI - 1),
                                )
                                nc.tensor.matmul(
                                    py1[:], lhs, w2t[:, il, H // 2 : H],
                                    start=(il == 0), stop=(il == GI - 1),
                                )
                            for hh, py in ((0, py0), (1, py1)):
                                hs = slice(hh * (H // 2), (hh + 1) * (H // 2))
                                if g == 0:
                                    nc.scalar.activation(
                                        ysb[tt][:, hs], py[:],
                                        mybir.ActivationFunctionType.Copy,
                                    )
                                else:
                                    nc.vector.tensor_add(
                                        ysb[tt][:, hs], ysb[tt][:, hs], py[:]
                                    )
                                if g == NG - 1:
                                    nc.sync.dma_start(
                                        out=y[t0 + tt * P : t0 + (tt + 1) * P, hs],
                                        in_=ysb[tt][:, hs],
                                    )

    nc.compile()
    return nc


def _get_nc():
    if "nc" not in _CACHE:
        _CACHE["nc"] = _build()
    return _CACHE["nc"]


def _pack_k(a, ktiles, dtype=np.float32):
    """[R, C] with R = ktiles*128 -> [128, ktiles, C] contiguous."""
    r, c = a.shape
    assert r == ktiles * P
    return np.ascontiguousarray(
        a.reshape(ktiles, P, c).transpose(1, 0, 2).astype(dtype)
    )


def _prep(x_sorted, w1, w2, expert_counts):
    x_sorted = np.ascontiguousarray(x_sorted, dtype=np.float32)
    w1 = np.asarray(w1, dtype=np.float32)
    w2 = np.asarray(w2, dtype=np.float32)
    counts = np.asarray(expert_counts, dtype=np.int64)

    n = x_sorted.shape[0]
    offsets = np.cumsum(counts)
    # per-token expert id, identical to reference's searchsorted
    eid = np.searchsorted(offsets, np.arange(n), side="right")

    in_maps = []
    row_idx = []
    for e in range(E):
        rows = np.nonzero(eid == e)[0]
        assert len(rows) <= T, f"expert {e} overflows capacity {T}"
        xe = np.zeros((T, H), dtype=np.float32)
        xe[: len(rows)] = x_sorted[rows]
        row_idx.append(rows)
        in_maps.append(
            {
                "xT": _pack_k(np.ascontiguousarray(xe.T), KH),
                "w1": _pack_k(w1[e], KH),
                "w2": _pack_k(w2[e], IB),
            }
        )
    return in_maps, row_idx


def kernel(x_sorted, w1, w2, expert_counts, local_expert_indices, **_unused):
    from concourse.bass_utils import run_bass_kernel_spmd

    n = np.asarray(x_sorted).shape[0]
    in_maps, row_idx = _prep(x_sorted, w1, w2, expert_counts)
    nc = _get_nc()

    res = run_bass_kernel_spmd(nc, in_maps, list(range(E))).results

    out = np.zeros((n, H), dtype=np.float32)
    for e in range(E):
        rows = row_idx[e]
        out[rows] = res[e]["y"][: len(rows)]
    return out
